# revision 9
# baseline (speedup 1.0000x reference)
"""EnhancedGCN (3-layer GCN + BN + ReLU) on 8 Trainium2 NeuronCores.

Strategy:
  - Nodes partitioned across 8 cores by dst range (graph parallel).
  - Host preprocesses the (fixed) graph: per-core edges packed densely into
    128-slot "tiles" (FFD bin-packing, <=4 dsts per tile), producing per-tile
    gather index columns and mask columns (mask value = dinv[dst] over the
    dst's slot segment).
  - Device per tile: indirect DMA gathers 128 rows (table pre-scaled by
    dinv[src]) -> PE matmul (lhsT=gathered [128,C], rhs=mask [128,4]) accumulates
    normalized aggregation into PSUM, channel-major.
  - Epilogue per 512-column region: linear W (PE), BN stats partials; BN bias
    is skipped (algebraically cancelled by BN). After each of layers 1-2:
    AllReduce of BN stats, fused BN+ReLU (scalar engine), scale by dinv[node],
    transpose back to node-major, AllGather the new table.
  - Layer 3 aggregates h2, multiplies W3, adds b3, writes per-core output.
Host unpermutes canonical column order back to natural node order.
"""
import numpy as np

N_NODES = 100000
N_CORES = 8
EPS = 1e-5
MAXD = 4          # max dsts per 128-slot tile
TILE = 128        # slots per gather tile
RTILES = 128      # tiles per region (=> 512 psum columns)


# ----------------------------------------------------------------- host plan
def _plan(edge_index, n_nodes, n_cores):
    src = edge_index[0].astype(np.int64)
    dst = edge_index[1].astype(np.int64)
    loops = np.arange(n_nodes, dtype=np.int64)
    src = np.concatenate([src, loops])
    dst = np.concatenate([dst, loops])
    deg = np.bincount(dst, minlength=n_nodes).astype(np.float64)
    dinv = (1.0 / np.sqrt(np.maximum(deg, 1.0))).astype(np.float32)

    per = n_nodes // n_cores
    # sort edges by dst once
    order = np.argsort(dst, kind="stable")
    src_s = src[order]
    dst_s = dst[order]
    starts = np.searchsorted(dst_s, np.arange(n_nodes))
    ends = np.searchsorted(dst_s, np.arange(n_nodes) + 1)

    cores = []
    max_tiles = 0
    for c in range(n_cores):
        lo, hi = c * per, (c + 1) * per
        nodes = np.arange(lo, hi)
        degs = (ends[lo:hi] - starts[lo:hi]).astype(np.int64)
        assert degs.max() <= TILE, f"degree {degs.max()} exceeds {TILE}"
        # FFD pack (degree desc) into tiles of <=128 slots, <=MAXD dsts
        order_d = np.argsort(-degs, kind="stable")
        tiles = []  # list of [slots_used, [node...]]
        open_tiles = []  # indices of tiles not yet full
        for oi in order_d:
            d = int(degs[oi])
            n = int(nodes[oi])
            placed = False
            for ti in open_tiles:
                t = tiles[ti]
                if t[0] + d <= TILE and len(t[1]) < MAXD:
                    t[0] += d
                    t[1].append(n)
                    if t[0] == TILE or len(t[1]) == MAXD:
                        open_tiles.remove(ti)
                    placed = True
                    break
            if not placed:
                tiles.append([d, [n]])
                if d < TILE:
                    open_tiles.append(len(tiles) - 1)
        cores.append((tiles, degs, nodes))
        max_tiles = max(max_tiles, len(tiles))

    T = ((max_tiles + RTILES - 1) // RTILES) * RTILES
    R = T // RTILES
    QC = MAXD * T                      # canonical columns per core

    R4 = 4 * R
    idx1 = np.zeros((n_cores, 128, T), np.int32)
    idx23 = np.zeros((n_cores, 128, T), np.int32)
    masks = np.zeros((n_cores, 128, MAXD * T), np.float32)
    dcol = np.zeros((n_cores, 1, QC), np.float32)
    node_of_col = np.full((n_cores, QC), -1, np.int64)
    canon = np.full(n_nodes, -1, np.int64)  # global canonical row per node

    for c in range(n_cores):
        tiles, degs, nodes = cores[c]
        for t, (_, tn) in enumerate(tiles):
            p = 0
            for m, n in enumerate(tn):
                q = MAXD * t + m
                node_of_col[c, q] = n
                canon[n] = c * QC + q
                dcol[c, 0, q] = dinv[n]
                d = int(deg[n])
                s0, s1 = starts[n], ends[n]
                idx1[c, p:p + d, t] = src_s[s0:s1]
                masks[c, p:p + d, q] = dinv[n]
                p += d
    assert (canon[: n_nodes] >= 0).all()
    for c in range(n_cores):
        idx23[c] = canon[idx1[c].reshape(-1)].reshape(128, T).astype(np.int32)
    dcol_pp = np.zeros((n_cores, 128, R4), np.float32)
    for c in range(n_cores):
        v = dcol[c, 0]  # [QC]
        dcol_pp[c] = v.reshape(R4, 128).T
    return dict(dinv=dinv, idx1=idx1, idx23=idx23, masks=masks, dcol=dcol,
                dcol_pp=dcol_pp, node_of_col=node_of_col, T=T, R=R, QC=QC,
                per=per)


# --------------------------------------------------------------- device build
def _build(plan, n_nodes, n_cores):
    import concourse.bacc as bacc
    import concourse.bass as bass
    import concourse.mybir as mybir
    import concourse.tile as tile
    from concourse.masks import make_identity

    f32 = mybir.dt.float32
    T, R, QC = plan["T"], plan["R"], plan["QC"]
    J0 = (n_nodes + 127) // 128          # x chunk cols per partition
    NPAD = 128 * J0
    ds = bass.ds

    nc = bacc.Bacc("TRN2", target_bir_lowering=False, debug=False,
                   num_devices=n_cores)
    # ---- inputs
    x_in = nc.dram_tensor("x_pad", [NPAD, 8], f32, kind="ExternalInput")
    dinv_nat = nc.dram_tensor("dinv_nat", [128, J0], f32, kind="ExternalInput")
    idx1_d = nc.dram_tensor("idx1", [128, T], mybir.dt.int32, kind="ExternalInput")
    idx23_d = nc.dram_tensor("idx23", [128, T], mybir.dt.int32, kind="ExternalInput")
    masks_d = nc.dram_tensor("masks", [128, MAXD * T], f32, kind="ExternalInput")
    dcol_d = nc.dram_tensor("dcol", [128, 4 * R], f32, kind="ExternalInput")
    w1_d = nc.dram_tensor("W1p", [8, 32], f32, kind="ExternalInput")
    w2_d = nc.dram_tensor("W2", [32, 32], f32, kind="ExternalInput")
    w3_d = nc.dram_tensor("W3", [32, 1], f32, kind="ExternalInput")
    gb_d = nc.dram_tensor("gb", [32, 4], f32, kind="ExternalInput")  # g1,b1,g2,b2
    b3_d = nc.dram_tensor("b3", [1, 1], f32, kind="ExternalInput")
    out_d = nc.dram_tensor("out", [1, R * 512], f32, kind="ExternalOutput")

    RG = list(range(n_cores))
    SROWS = ((R + 3) // 4) * 512         # stash cols per partition band

    with tile.TileContext(nc) as tc:
        with (
            tc.tile_pool(name="const", bufs=1) as cpool,
            tc.tile_pool(name="sb", bufs=2) as pool,
            tc.tile_pool(name="gbuf", bufs=2) as gpool,
            tc.tile_pool(name="ps", bufs=2, space="PSUM") as psum,
            tc.tile_pool(name="ps2", bufs=2, space="PSUM") as psum2,
            tc.tile_pool(name="dram", bufs=1, space="DRAM") as dpool,
        ):
            ident = cpool.tile([128, 128], f32)
            make_identity(nc, ident[:])
            zlhs = cpool.tile([128, 32], f32)
            nc.vector.memset(zlhs[:], 0.0)
            w1_t = cpool.tile([8, 32], f32)
            nc.sync.dma_start(out=w1_t[:], in_=w1_d[:, :])
            w2_t = cpool.tile([32, 32], f32)
            nc.sync.dma_start(out=w2_t[:], in_=w2_d[:, :])
            w3_t = cpool.tile([32, 1], f32)
            nc.sync.dma_start(out=w3_t[:], in_=w3_d[:, :])
            gb_t = cpool.tile([32, 4], f32)
            nc.sync.dma_start(out=gb_t[:], in_=gb_d[:, :])
            b3_t = cpool.tile([1, 1], f32)
            nc.sync.dma_start(out=b3_t[:], in_=b3_d[:, :])
            eps_t = cpool.tile([32, 1], f32)
            nc.vector.memset(eps_t[:], float(EPS))
            idx1_sb = cpool.tile([128, T], mybir.dt.int32)
            nc.sync.dma_start(out=idx1_sb[:], in_=idx1_d[:, :])
            idx23_sb = cpool.tile([128, T], mybir.dt.int32)
            nc.sync.dma_start(out=idx23_sb[:], in_=idx23_d[:, :])
            stash = cpool.tile([128, SROWS], f32)
            dcol_sb = cpool.tile([128, 4 * R], f32)
            nc.sync.dma_start(out=dcol_sb[:], in_=dcol_d[:, :])

            # ---- tables (DRAM)
            t1 = dpool.tile([NPAD, 8], f32, name="t1")
            t2loc = dpool.tile([QC, 32], f32, name="t2loc")
            t2glob = dpool.tile([n_cores * QC, 32], f32, name="t2glob",
                                addr_space="Shared")
            t3loc = dpool.tile([QC, 32], f32, name="t3loc")
            t3glob = dpool.tile([n_cores * QC, 32], f32, name="t3glob",
                                addr_space="Shared")

            # ---- phase 0: T1 = x * dinv[n] (node-major, 8ch)
            xs = cpool.tile([128, J0 * 8], f32, name="xs")
            nc.sync.dma_start(out=xs[:], in_=x_in[:, :].rearrange("(p j) c -> p (j c)", p=128))
            dv = cpool.tile([128, J0], f32, name="dv")
            nc.sync.dma_start(out=dv[:], in_=dinv_nat[:, :])
            nc.vector.tensor_mul(
                out=xs[:].rearrange("p (j c) -> p j c", c=8),
                in0=xs[:].rearrange("p (j c) -> p j c", c=8),
                in1=dv[:].rearrange("p (j o) -> p j o", o=1).to_broadcast([128, J0, 8]))
            nc.sync.dma_start(out=t1[:, :].rearrange("(p j) c -> p (j c)", p=128), in_=xs[:])

            # ---- per layer
            def agg_layer(idx_sb, table_ap, cin, stats_acc=None, wt=None,
                          l3=False):
                """runs all regions; returns nothing (writes stash or out)."""
                for r in range(R):
                    mreg = pool.tile([128, MAXD * RTILES], f32, tag="mreg",
                                     name=f"mreg")
                    nc.sync.dma_start(
                        out=mreg[:],
                        in_=masks_d[:, r * MAXD * RTILES:(r + 1) * MAXD * RTILES])
                    acc = psum.tile([cin, 512], f32, tag="acc", name="acc")
                    nc.tensor.matmul(acc[:], zlhs[:, :cin], mreg[:],
                                     start=True, stop=False,
                                     skip_group_check=True)
                    gs = [gpool.tile([128, cin], f32, tag=f"g{u}", name=f"g{u}")
                          for u in range(16)]
                    iA = gpool.tile([128, 8], mybir.dt.int32, tag="iA", name="iA")
                    iB = gpool.tile([128, 8], mybir.dt.int32, tag="iB", name="iB")
                    base = r * RTILES
                    with tc.For_i(0, 8, 1) as i:
                        for half, ib in ((0, iA), (1, iB)):
                            nc.vector.tensor_copy(
                                out=ib[:],
                                in_=idx_sb[:, ds(base + i * 16 + half * 8, 8)])
                            for u in range(8):
                                uu = half * 8 + u
                                g = gs[uu]
                                nc.gpsimd.indirect_dma_start(
                                    out=g[:], out_offset=None, in_=table_ap,
                                    in_offset=bass.IndirectOffsetOnAxis(
                                        ap=ib[:, u:u + 1], axis=0))
                                nc.tensor.matmul(
                                    acc[:, ds(i * 64 + uu * 4, 4)], g[:],
                                    mreg[:, ds(i * 64 + uu * 4, 4)],
                                    start=False, stop=False,
                                    skip_group_check=True)
                    # epilogue
                    agg_sb = pool.tile([cin, 512], f32, tag="aggsb", name="agg_sb")
                    nc.scalar.copy(out=agg_sb[:], in_=acc[:])
                    if not l3:
                        hps = psum2.tile([32, 512], f32, tag="hps", name="hps")
                        nc.tensor.matmul(hps[:], wt[:], agg_sb[:],
                                         start=True, stop=True,
                                         skip_group_check=True)
                        sl = stash[32 * (r % 4):32 * (r % 4) + 32,
                                   512 * (r // 4):512 * (r // 4) + 512]
                        nc.scalar.copy(out=sl, in_=hps[:])
                        s1 = pool.tile([32, 1], f32, tag="s1t", name="s1")
                        nc.vector.reduce_sum(out=s1[:], in_=sl,
                                             axis=mybir.AxisListType.X)
                        sq = pool.tile([32, 512], f32, tag="sqt", name="sq")
                        nc.scalar.square(out=sq[:], in_=sl)
                        s2 = pool.tile([32, 1], f32, tag="s2t", name="s2")
                        nc.vector.reduce_sum(out=s2[:], in_=sq[:],
                                             axis=mybir.AxisListType.X)
                        nc.vector.tensor_add(out=stats_acc[:, 0:1],
                                             in0=stats_acc[:, 0:1], in1=s1[:])
                        nc.vector.tensor_add(out=stats_acc[:, 1:2],
                                             in0=stats_acc[:, 1:2], in1=s2[:])
                    else:
                        ops = psum2.tile([1, 512], f32, tag="ops", name="ops")
                        nc.tensor.matmul(ops[:], wt[:], agg_sb[:],
                                         start=True, stop=True,
                                         skip_group_check=True)
                        ot = pool.tile([1, 512], f32, tag="ot", name="ot")
                        nc.scalar.activation(out=ot[:], in_=ops[:],
                                             func=mybir.ActivationFunctionType.Identity,
                                             bias=b3_t[:, 0:1], scale=1.0)
                        nc.sync.dma_start(out=out_d[:, r * 512:(r + 1) * 512],
                                          in_=ot[:])

            def bn_pass(stats_acc, gi, tloc, tglob, inv_n):
                # AllReduce stats
                sin = dpool.tile([32, 2], f32, name=f"sin{gi}")
                sout = dpool.tile([32, 2], f32, name=f"sout{gi}",
                                  addr_space="Shared")
                nc.sync.dma_start(out=sin[:, :], in_=stats_acc[:])
                nc.gpsimd.collective_compute(
                    "AllReduce", mybir.AluOpType.add, replica_groups=[RG],
                    ins=[sin[:, :].opt()], outs=[sout[:, :].opt()])
                st = pool.tile([32, 2], f32, tag="stt", name="st")
                nc.sync.dma_start(out=st[:], in_=sout[:, :])
                mean = pool.tile([32, 1], f32, tag="bn1", name="mean")
                nc.scalar.mul(out=mean[:], in_=st[:, 0:1], mul=inv_n)
                ex2 = pool.tile([32, 1], f32, tag="bn2", name="ex2")
                nc.scalar.mul(out=ex2[:], in_=st[:, 1:2], mul=inv_n)
                m2 = pool.tile([32, 1], f32, tag="bn3", name="m2")
                nc.scalar.square(out=m2[:], in_=mean[:])
                var = pool.tile([32, 1], f32, tag="bn4", name="var")
                nc.vector.tensor_tensor(out=var[:], in0=ex2[:], in1=m2[:],
                                        op=mybir.AluOpType.subtract)
                sd = pool.tile([32, 1], f32, tag="bn5", name="sd")
                nc.scalar.activation(out=sd[:], in_=var[:],
                                     func=mybir.ActivationFunctionType.Sqrt,
                                     bias=eps_t[:, 0:1], scale=1.0)
                inv = pool.tile([32, 1], f32, tag="bn6", name="inv")
                nc.vector.reciprocal(out=inv[:], in_=sd[:])
                A = pool.tile([32, 1], f32, tag="bn7", name="A")
                nc.vector.tensor_mul(out=A[:], in0=gb_t[:, 2 * gi:2 * gi + 1],
                                     in1=inv[:])
                mA = pool.tile([32, 1], f32, tag="bn8", name="mA")
                nc.vector.tensor_mul(out=mA[:], in0=mean[:], in1=A[:])
                B = pool.tile([32, 1], f32, tag="bn9", name="B")
                nc.vector.tensor_tensor(out=B[:], in0=gb_t[:, 2 * gi + 1:2 * gi + 2],
                                        in1=mA[:], op=mybir.AluOpType.subtract)
                # pass B: BN+ReLU, scale by dinv, transpose, store, allgather
                for r in range(R):
                    sl = stash[32 * (r % 4):32 * (r % 4) + 32,
                               512 * (r // 4):512 * (r // 4) + 512]
                    un = pool.tile([32, 512], f32, tag="un", name="un")
                    nc.scalar.activation(out=un[:], in_=sl,
                                         func=mybir.ActivationFunctionType.Relu,
                                         bias=B[:, 0:1], scale=A[:, 0:1])
                    tsb = pool.tile([128, 4 * 32], f32, tag="tsb", name="tsb")
                    for k in range(4):
                        tp = psum2.tile([128, 32], f32, tag="tp", name="tp")
                        nc.tensor.transpose(tp[:], un[:, 128 * k:128 * k + 128],
                                            ident[:32, :32])
                        nc.scalar.mul(out=tsb[:, 32 * k:32 * k + 32], in_=tp[:],
                                      mul=dcol_sb[:, 4 * r + k:4 * r + k + 1])
                    nc.sync.dma_start(
                        out=tloc[r * 512:(r + 1) * 512, :].rearrange(
                            "(k p) c -> p k c", p=128),
                        in_=tsb[:].rearrange("p (k c) -> p k c", k=4))
                nc.gpsimd.collective_compute(
                    "AllGather", mybir.AluOpType.bypass, replica_groups=[RG],
                    ins=[tloc[:, :].opt()], outs=[tglob[:, :].opt()])

            # L1
            stats1 = cpool.tile([32, 2], f32)
            nc.vector.memset(stats1[:], 0.0)
            agg_layer(idx1_sb, t1[:, :], 8, stats_acc=stats1, wt=w1_t)
            bn_pass(stats1, 0, t2loc, t2glob, 1.0 / n_nodes)
            # L2
            stats2 = cpool.tile([32, 2], f32)
            nc.vector.memset(stats2[:], 0.0)
            agg_layer(idx23_sb, t2glob[:, :], 32, stats_acc=stats2, wt=w2_t)
            bn_pass(stats2, 1, t3loc, t3glob, 1.0 / n_nodes)
            # L3
            agg_layer(idx23_sb, t3glob[:, :], 32, wt=w3_t, l3=True)

    nc.compile()
    return nc


# ------------------------------------------------------------------- kernel
def kernel(x, edge_index, W1, b1, gamma1, beta1, W2, b2, gamma2, beta2, W3, b3):
    from concourse.bass_utils import run_bass_kernel_spmd

    x = np.asarray(x, np.float32)
    edge_index = np.asarray(edge_index)
    n_nodes = x.shape[0]
    plan = _plan(edge_index, n_nodes, N_CORES)
    nc = _build(plan, n_nodes, N_CORES)

    J0 = (n_nodes + 127) // 128
    NPAD = 128 * J0
    x_pad = np.zeros((NPAD, 8), np.float32)
    x_pad[:n_nodes, :5] = x
    dinv_nat = np.zeros((128, J0), np.float32)
    dinv_nat.reshape(-1)[:n_nodes] = plan["dinv"]
    # note: reshape(-1) is row-major [128, J0] -> index p*J0+j matches layout
    W1p = np.zeros((8, 32), np.float32)
    W1p[:5] = np.asarray(W1, np.float32)
    gb = np.stack([np.asarray(gamma1, np.float32), np.asarray(beta1, np.float32),
                   np.asarray(gamma2, np.float32), np.asarray(beta2, np.float32)],
                  axis=1)

    in_maps = []
    for c in range(N_CORES):
        in_maps.append({
            "x_pad": x_pad,
            "dinv_nat": dinv_nat,
            "idx1": plan["idx1"][c],
            "idx23": plan["idx23"][c],
            "masks": plan["masks"][c],
            "dcol": plan["dcol_pp"][c],
            "W1p": W1p,
            "W2": np.asarray(W2, np.float32),
            "W3": np.asarray(W3, np.float32),
            "gb": gb,
            "b3": np.asarray(b3, np.float32).reshape(1, 1),
        })
    res = run_bass_kernel_spmd(nc, in_maps, core_ids=list(range(N_CORES)))

    out = np.zeros((n_nodes, 1), np.float32)
    for c in range(N_CORES):
        vals = res.results[c]["out"].reshape(-1)
        noc = plan["node_of_col"][c]
        m = noc >= 0
        out[noc[m], 0] = vals[:len(noc)][m]
    return out
